# revision 25
# baseline (speedup 1.0000x reference)
"""AttnBlock6 Trainium2 kernel (Bass/Tile, 8 NeuronCores).

Math (per sample):
  xn = GroupNorm1(x);  q,k,v = 1x1conv(xn);  wm = softmax(qf^T kf / 256)
  hp = vf @ wm^T   (patch attention over 196-wide chunks, contraction 65536)
  pooled 8x8 -> qg,kg,vg [64,3136]; wg = softmax(qg^T kg / 8); hg = vg @ wg^T
  out = x + proj( 0.75*hp + 0.25*up8(hg) )

Distribution: cores 0-3 -> sample 0, cores 4-7 -> sample 1; each core owns 112
image rows (= 256 patch chunks, 784 pooled positions). Two tiny collectives
per 4-core group: AllReduce of the 196x196 gram matrix, AllGather of pooled x.

Host-side folding: per-sample GroupNorm mean/inv-std are computed on the host
(O(N) numpy) and folded into per-sample q/k/v weight matrices, so the device
never materializes xn. Softmax scales (1/256, 1/8) and the 0.75/0.25 combine
weights are folded into weights/normalizers. v-bias contributes exactly c_v to
the combined h (softmax rows sum to 1), folded in once via the hg term.

Device layout: partition p = 64*half + channel; free = within-half position
(row-major over the core's 56-row half). Block-diagonal duplicated weights
give full K=128 matmul contractions.
"""

import numpy as np

import concourse.bacc as bacc
import concourse.bass as bass
import concourse.tile as tile
from concourse import mybir
from concourse.masks import make_identity

C = 64
SIZE = 448
P2 = 196
TG = 56
POOL = 8
EPS = 1e-5
B = 2

NCORES = 8
GROUP = 4                  # cores per sample
ROWS = SIZE // GROUP       # 112 image rows per core
F = ROWS * SIZE // 2       # 25088 free elems per half
SB = 3136                  # super-block: 7 rows = 16 chunks per half
NSB = F // SB              # 8
PPL = 784                  # pooled positions per core (2 * 7 * 56)
PG = 3136                  # pooled positions per sample (56*56)
KT = 112                   # pooled k-tile (fits rank-contiguous AGed layout)
NKT = PG // KT             # 28

FP32 = mybir.dt.float32
FP32R = mybir.dt.float32r
BF16 = mybir.dt.bfloat16
AF = mybir.ActivationFunctionType
ALU = mybir.AluOpType

_CACHE = {}


def _build_bass(reps=1):
    nc = bacc.Bacc("TRN2", target_bir_lowering=False, debug=False,
                   num_devices=NCORES)

    x2 = nc.dram_tensor("x2", [128, F], FP32, kind="ExternalInput")
    w2q = nc.dram_tensor("w2q", [128, 128], BF16, kind="ExternalInput")
    w2k = nc.dram_tensor("w2k", [128, 128], BF16, kind="ExternalInput")
    w2v = nc.dram_tensor("w2v", [128, 128], BF16, kind="ExternalInput")
    w2p = nc.dram_tensor("w2p", [128, 128], BF16, kind="ExternalInput")
    cq2 = nc.dram_tensor("cq2", [128, 1], FP32, kind="ExternalInput")
    ck2 = nc.dram_tensor("ck2", [128, 1], FP32, kind="ExternalInput")
    cv2 = nc.dram_tensor("cv2", [128, 1], FP32, kind="ExternalInput")
    agq = nc.dram_tensor("agq", [64, 64], BF16, kind="ExternalInput")
    agk = nc.dram_tensor("agk", [64, 64], BF16, kind="ExternalInput")
    agv = nc.dram_tensor("agv", [64, 64], BF16, kind="ExternalInput")
    cgq = nc.dram_tensor("cgq", [64, 1], FP32, kind="ExternalInput")
    cgk = nc.dram_tensor("cgk", [64, 1], FP32, kind="ExternalInput")
    y2 = nc.dram_tensor("y2", [128, F], FP32, kind="ExternalOutput")

    with tile.TileContext(nc) as tc:
        for _ in range(reps):
            _emit(nc, tc, x2, w2q, w2k, w2v, w2p, cq2, ck2, cv2,
                  agq, agk, agv, cgq, cgk, y2)
    nc.finalize()
    return nc


def _ap(t, dims, offset=0):
    """Manual strided view of a DRAM tile: dims = [[step, count], ...]."""
    return bass.AP(tensor=t.tensor, offset=t.offset + offset, ap=list(dims))


def _emit(nc, tc, x2, w2q, w2k, w2v, w2p, cq2, ck2, cv2,
          agq, agk, agv, cgq, cgk, y2):
    import contextlib
    ctx = contextlib.ExitStack()
    with ctx:
        singles = ctx.enter_context(tc.tile_pool(name="singles", bufs=1))
        dram = ctx.enter_context(tc.tile_pool(name="dram", bufs=1, space="DRAM"))

        # ---- load constants ----
        W2q = singles.tile([128, 128], BF16)
        W2k = singles.tile([128, 128], BF16)
        W2v = singles.tile([128, 128], BF16)
        W2p = singles.tile([128, 128], BF16)
        for t, src in ((W2q, w2q), (W2k, w2k), (W2v, w2v), (W2p, w2p)):
            nc.gpsimd.dma_start(out=t[:], in_=src[:, :])
        Cq = singles.tile([128, 1], FP32)
        Ck = singles.tile([128, 1], FP32)
        Cv = singles.tile([128, 1], FP32)
        for t, src in ((Cq, cq2), (Ck, ck2), (Cv, cv2)):
            nc.sync.dma_start(out=t[:], in_=src[:, :])
        Agq = singles.tile([64, 64], BF16)
        Agk = singles.tile([64, 64], BF16)
        Agv = singles.tile([64, 64], BF16)
        for t, src in ((Agq, agq), (Agk, agk), (Agv, agv)):
            nc.sync.dma_start(out=t[:], in_=src[:, :])
        Cgq = singles.tile([64, 1], FP32)
        Cgk = singles.tile([64, 1], FP32)
        for t, src in ((Cgq, cgq), (Cgk, cgk)):
            nc.sync.dma_start(out=t[:], in_=src[:, :])
        ident = singles.tile([98, 98], FP32)
        make_identity(nc, ident)

        # ---- persistent SBUF ----
        XB = singles.tile([128, F], BF16)         # bf16 x, 2-half layout
        poolw = singles.tile([128, 56, 56], FP32)  # w-pooled sums
        pool2 = singles.tile([128, 392], FP32)     # 8x8-pooled sums (2-half)
        pool2b = singles.tile([128, 392], BF16)

        # DRAM scratch / collective buffers
        cc_m_in = dram.tile([P2, P2], BF16)
        cc_m_out = dram.tile([GROUP, P2, P2], BF16)
        cc_p_in = dram.tile([64, PPL], BF16)
        cc_p_out = dram.tile([GROUP * 64, PPL], BF16)
        hg_dram = dram.tile([64, PPL], FP32)
        rec_dram = dram.tile([1, PPL], FP32)

        # =========== PASS A ===========
        mps_a = None
        mps_b = None
        with (
            tc.tile_pool(name="apsum", bufs=3, space="PSUM") as apsum,
            tc.tile_pool(name="mpsum", bufs=1, space="PSUM") as mpsum,
            tc.tile_pool(name="aqk", bufs=4) as aqk,
            tc.tile_pool(name="apool", bufs=2) as apool,
            tc.tile_pool(name="axt", bufs=3) as axt,
        ):
            mps_a = mpsum.tile([98, P2], FP32, tag="mps_a")
            mps_b = mpsum.tile([98, P2], FP32, tag="mps_b")
            nblk = 0
            for sb in range(NSB):
                o = sb * SB
                Xt = axt.tile([128, SB], FP32, tag="xt")
                nc.sync.dma_start(out=Xt[:], in_=x2[:, o:o + SB])
                xb = XB[:, o:o + SB]
                nc.gpsimd.tensor_copy(out=xb, in_=Xt[:])
                # pooling (from bf16 x): sb0-5 one DVE strided reduce,
                # sb6-7 gpsimd 3-op tree
                pws = poolw[:, sb * 7:sb * 7 + 7, :]
                if sb < 6:
                    nc.vector.reduce_sum(
                        out=pws,
                        in_=xb.rearrange("p (r c ew) -> p r c ew", r=7, ew=8),
                        axis=mybir.AxisListType.X)
                else:
                    x2v = xb.rearrange("p (q e) -> p q e", e=2)
                    pt = apool.tile([128, 1568], FP32, tag="pt")
                    ptv = pt[:].rearrange("p (q e) -> p q e", e=2)
                    nc.gpsimd.tensor_tensor(pt[:], x2v[:, :, 0], x2v[:, :, 1],
                                            ALU.add)
                    pu = apool.tile([128, 784], FP32, tag="pu")
                    puv = pu[:].rearrange("p (q e) -> p q e", e=2)
                    nc.gpsimd.tensor_tensor(pu[:], ptv[:, :, 0], ptv[:, :, 1],
                                            ALU.add)
                    nc.gpsimd.tensor_tensor(pws, puv[:, :, 0], puv[:, :, 1],
                                            ALU.add)
                # q/k projections + gram accumulation (784-col blocks)
                for j in range(SB // 784):
                    a = o + j * 784
                    qp = apsum.tile([128, 2, 512], FP32, tag="qkp")
                    kp = apsum.tile([128, 2, 512], FP32, tag="qkp")
                    for u in range(2):
                        xr = XB[:, a + u * 392:a + u * 392 + 392]
                        nc.tensor.matmul(qp[:, u, 0:392], W2q[:], xr,
                                         skip_group_check=True)
                        nc.tensor.matmul(kp[:, u, 0:392], W2k[:], xr,
                                         skip_group_check=True)
                    qs = aqk.tile([128, 784], BF16, tag="qs")
                    ks = aqk.tile([128, 784], BF16, tag="ks")
                    qsv = qs[:].rearrange("p (u c) -> p u c", u=2)
                    ksv = ks[:].rearrange("p (u c) -> p u c", u=2)
                    if j % 2 == 0:
                        nc.scalar.activation(qsv, qp[:, :, 0:392], AF.Identity,
                                             bias=Cq[:, 0:1])
                        nc.vector.tensor_scalar(ksv, kp[:, :, 0:392],
                                                Ck[:, 0:1], None, op0=ALU.add)
                    else:
                        nc.vector.tensor_scalar(qsv, qp[:, :, 0:392],
                                                Cq[:, 0:1], None, op0=ALU.add)
                        nc.scalar.activation(ksv, kp[:, :, 0:392], AF.Identity,
                                             bias=Ck[:, 0:1])
                    for cc in range(4):
                        q1 = qs[:, cc * 196:cc * 196 + 196]
                        k1 = ks[:, cc * 196:cc * 196 + 196]
                        first = nblk == 0
                        last = nblk == 127
                        nc.tensor.matmul(mps_a[:], q1[:, 0:98], k1,
                                         start=first, stop=last)
                        nc.tensor.matmul(mps_b[:], q1[:, 98:196], k1,
                                         start=first, stop=last)
                        nblk += 1
            # h-pool on gpsimd (3-op tree) + ship pooled x -> AllGather
            hv = poolw[:].rearrange("p (r e) c -> p r e c", e=2)
            ht = apool.tile([128, 28, 56], FP32, tag="pt")
            nc.gpsimd.tensor_tensor(ht[:], hv[:, :, 0, :], hv[:, :, 1, :],
                                    ALU.add)
            htv = ht[:].rearrange("p (r e) c -> p r e c", e=2)
            hu = apool.tile([128, 14, 56], FP32, tag="pu")
            nc.gpsimd.tensor_tensor(hu[:], htv[:, :, 0, :], htv[:, :, 1, :],
                                    ALU.add)
            huv = hu[:].rearrange("p (r e) c -> p r e c", e=2)
            nc.gpsimd.tensor_tensor(
                pool2[:].rearrange("p (r c) -> p r c", r=7),
                huv[:, :, 0, :], huv[:, :, 1, :], ALU.add)
            nc.gpsimd.tensor_copy(out=pool2b[:], in_=pool2[:])
            nc.gpsimd.dma_start(
                out=_ap(cc_p_in, [[392, 2], [PPL, 64], [1, 392]]),
                in_=pool2b[:])
            nc.gpsimd.collective_compute(
                "AllGather", ALU.bypass,
                replica_groups=[[0, 1, 2, 3], [4, 5, 6, 7]],
                ins=[cc_p_in.opt()], outs=[cc_p_out.opt()])

            # gram eviction -> AllReduce
            msb_a = aqk.tile([98, P2], BF16, tag="msb_a")
            msb_b = aqk.tile([98, P2], BF16, tag="msb_b")
            nc.scalar.copy(msb_a[:], mps_a[:])
            nc.scalar.copy(msb_b[:], mps_b[:])
            nc.gpsimd.dma_start(out=cc_m_in[0:98, :], in_=msb_a[:])
            nc.gpsimd.dma_start(out=cc_m_in[98:196, :], in_=msb_b[:])
            nc.gpsimd.collective_compute(
                "AllGather", ALU.bypass,
                replica_groups=[[0, 1, 2, 3], [4, 5, 6, 7]],
                ins=[cc_m_in.opt()], outs=[cc_m_out.opt()])

        # =========== global pooled attention ===========
        hgc2 = singles.tile([128, 392], FP32)
        with (
            tc.tile_pool(name="gp", bufs=1) as gp,
            tc.tile_pool(name="gwge", bufs=3) as gwge,
            tc.tile_pool(name="gps", bufs=2, space="PSUM") as gps,
            tc.tile_pool(name="ghps", bufs=1, space="PSUM") as ghps,
        ):
            kgsrc = gp.tile([64, PG], BF16, tag="kgsrc")
            nc.sync.dma_start(
                out=kgsrc[:].rearrange("c (r i) -> c r i", r=GROUP),
                in_=_ap(cc_p_out, [[PPL, 64], [64 * PPL, GROUP], [1, PPL]]))
            # qg for my local positions (pooled x back in [64, 784] layout)
            qsrc = gp.tile([64, PPL], BF16, tag="qsrc")
            nc.sync.dma_start(out=qsrc[:], in_=cc_p_in[:, :])
            qgb = gp.tile([64, PPL], BF16, tag="qgb")
            for h in range(2):
                qp = gps.tile([64, 392], FP32, tag="gsm")
                nc.tensor.matmul(qp[:], Agq[:],
                                 qsrc[:, h * 392:h * 392 + 392])
                nc.scalar.activation(qgb[:, h * 392:h * 392 + 392], qp[:],
                                     AF.Identity, bias=Cgq[:, 0:1])
            # kg for all positions of the sample
            kgb = gp.tile([64, PG], BF16, tag="kgb")
            for j in range(2 * GROUP):
                kp = gps.tile([64, 392], FP32, tag="gsm")
                nc.tensor.matmul(kp[:], Agk[:], kgsrc[:, j * 392:(j + 1) * 392])
                nc.scalar.activation(kgb[:, j * 392:(j + 1) * 392], kp[:],
                                     AF.Identity, bias=Cgk[:, 0:1])
            # vgT tiles [112, 65] (last col = ones for the denominator row)
            vgT = gp.tile([112, NKT, 65], BF16, tag="vgT")
            for kt in range(NKT):
                vp = gps.tile([112, 64], FP32, tag="gsm")
                nc.tensor.matmul(vp[:], kgsrc[:, kt * KT:(kt + 1) * KT], Agv[:])
                nc.scalar.copy(vgT[:, kt, 0:64], vp[:])
                nc.vector.memset(vgT[:, kt, 64:65], 1.0)
            # stream k-tiles: wgT logits -> exp -> accumulate hg + denom
            hg0 = ghps.tile([65, 392], FP32, tag="hg0")
            hg1 = ghps.tile([65, 392], FP32, tag="hg1")
            for kt in range(NKT):
                wgp = gps.tile([112, PPL], FP32, tag="wgp")
                for n in range(2):
                    nc.tensor.matmul(wgp[:, n * 392:n * 392 + 392],
                                     kgb[:, kt * KT:(kt + 1) * KT],
                                     qgb[:, n * 392:n * 392 + 392])
                wge = gwge.tile([112, PPL], BF16, tag="wge")
                nc.scalar.activation(wge[:], wgp[:], AF.Exp)
                for n, hgp in ((0, hg0), (1, hg1)):
                    nc.tensor.matmul(hgp[:], vgT[:, kt, :],
                                     wge[:, n * 392:n * 392 + 392],
                                     start=(kt == 0), stop=(kt == NKT - 1))
            # normalize + rearrange to (half,c) layout via DRAM bounce
            hgA = gp.tile([64, PPL], FP32, tag="hgA")
            rec = gp.tile([1, PPL], FP32, tag="rec")
            for n, hgp in ((0, hg0), (1, hg1)):
                nc.scalar.copy(hgA[:, n * 392:n * 392 + 392], hgp[0:64, :])
                nc.scalar.copy(rec[:, n * 392:n * 392 + 392], hgp[64:65, :])
            nc.vector.reciprocal(out=rec[:], in_=rec[:])
            nc.vector.tensor_scalar_mul(rec[:], rec[:], 0.25)
            nc.sync.dma_start(out=hg_dram[:], in_=hgA[:])
            nc.sync.dma_start(out=rec_dram[:], in_=rec[:])
            hgB = gp.tile([128, 392], FP32, tag="hgB")
            nc.sync.dma_start(out=hgB[:],
                              in_=_ap(hg_dram, [[392, 2], [PPL, 64], [1, 392]]))
            rec2 = gp.tile([128, 392], FP32, tag="rec2")
            nc.sync.dma_start(
                out=rec2[:],
                in_=_ap(rec_dram, [[392, 2], [0, 64], [1, 392]]))
            nc.vector.tensor_tensor(hgc2[:], hgB[:], rec2[:], ALU.mult)
            nc.vector.tensor_scalar(hgc2[:], hgc2[:], Cv[:, 0:1], None,
                                    op0=ALU.add)

        # =========== local softmax of wm; build wmT (bf16, x0.75) ===========
        wmT_a = singles.tile([98, P2], BF16)   # k in [0,98)
        wmT_b = singles.tile([98, P2], BF16)   # k in [98,196)
        with (
            tc.tile_pool(name="wmp", bufs=1) as wmp,
            tc.tile_pool(name="wmps", bufs=2, space="PSUM") as wmps,
        ):
            m0 = wmp.tile([98, GROUP, P2], BF16, tag="m0")
            m1 = wmp.tile([98, GROUP, P2], BF16, tag="m1")
            nc.sync.dma_start(
                out=m0[:], in_=_ap(cc_m_out, [[P2, 98], [P2 * P2, GROUP],
                                              [1, P2]]))
            nc.sync.dma_start(
                out=m1[:], in_=_ap(cc_m_out, [[P2, 98], [P2 * P2, GROUP],
                                              [1, P2]], offset=98 * P2))
            ms0 = wmp.tile([98, P2], FP32, tag="ms0")
            ms1 = wmp.tile([98, P2], FP32, tag="ms1")
            nc.vector.tensor_tensor(ms0[:], m0[:, 0, :], m0[:, 1, :], ALU.add)
            nc.vector.tensor_tensor(ms0[:], ms0[:], m0[:, 2, :], ALU.add)
            nc.vector.tensor_tensor(ms0[:], ms0[:], m0[:, 3, :], ALU.add)
            nc.vector.tensor_tensor(ms1[:], m1[:, 0, :], m1[:, 1, :], ALU.add)
            nc.vector.tensor_tensor(ms1[:], ms1[:], m1[:, 2, :], ALU.add)
            nc.vector.tensor_tensor(ms1[:], ms1[:], m1[:, 3, :], ALU.add)
            e0 = wmp.tile([98, P2], FP32, tag="e0")
            e1 = wmp.tile([98, P2], FP32, tag="e1")
            nc.scalar.activation(e0[:], ms0[:], AF.Exp)
            nc.scalar.activation(e1[:], ms1[:], AF.Exp)
            s0 = wmp.tile([98, 1], FP32, tag="s0")
            s1 = wmp.tile([98, 1], FP32, tag="s1")
            nc.vector.reduce_sum(out=s0[:], in_=e0[:], axis=mybir.AxisListType.X)
            nc.vector.reduce_sum(out=s1[:], in_=e1[:], axis=mybir.AxisListType.X)
            nc.vector.reciprocal(out=s0[:], in_=s0[:])
            nc.vector.reciprocal(out=s1[:], in_=s1[:])
            nc.vector.tensor_scalar_mul(s0[:], s0[:], 0.75)
            nc.vector.tensor_scalar_mul(s1[:], s1[:], 0.75)
            wn0 = wmp.tile([98, P2], FP32, tag="wn0")
            wn1 = wmp.tile([98, P2], FP32, tag="wn1")
            nc.vector.tensor_scalar(wn0[:], e0[:], s0[:, 0:1], None, op0=ALU.mult)
            nc.vector.tensor_scalar(wn1[:], e1[:], s1[:, 0:1], None, op0=ALU.mult)
            for (dst, srcs) in ((wmT_a, (wn0[:, 0:98], wn1[:, 0:98])),
                                (wmT_b, (wn0[:, 98:196], wn1[:, 98:196]))):
                for half, src in enumerate(srcs):
                    tp = wmps.tile([98, 98], FP32, tag="tp")
                    nc.tensor.transpose(tp[:], src, ident[:])
                    nc.scalar.copy(dst[:, half * 98:half * 98 + 98], tp[:])

        # =========== PASS B ===========
        with (
            tc.tile_pool(name="bvt", bufs=4) as bvt,
            tc.tile_pool(name="bh", bufs=2) as bh,
            tc.tile_pool(name="bout", bufs=4) as bout,
            tc.tile_pool(name="bvps", bufs=2, space="PSUM") as bvps,
            tc.tile_pool(name="bhps", bufs=2, space="PSUM") as bhps,
            tc.tile_pool(name="bpps", bufs=2, space="PSUM") as bpps,
        ):
            for sb in range(NSB):
                o = sb * SB
                h_sb = bh.tile([128, SB], BF16, tag="h")
                hps = []
                for t in range(SB // 392):
                    a = t * 392
                    vp = bvps.tile([98, 512], FP32, tag="vp")
                    for u in range(4):
                        nc.tensor.matmul(vp[:, u * 128:u * 128 + 128],
                                         XB[:, o + a + u * 98:o + a + u * 98 + 98],
                                         W2v[:], skip_group_check=True)
                    vt = bvt.tile([98, 512], BF16, tag="vt")
                    nc.scalar.copy(vt[:], vp[:])
                    hp = bhps.tile([128, 392], FP32, tag="hp")
                    for cc in range(2):
                        u = 2 * cc
                        nc.tensor.matmul(hp[:, cc * 196:cc * 196 + 196],
                                         vt[:, u * 128:u * 128 + 128], wmT_a[:],
                                         start=True, stop=False,
                                         skip_group_check=True)
                        nc.tensor.matmul(hp[:, cc * 196:cc * 196 + 196],
                                         vt[:, (u + 1) * 128:(u + 1) * 128 + 128],
                                         wmT_b[:],
                                         start=False, stop=True,
                                         skip_group_check=True)
                    hps.append(hp)
                # combine h = hp + hgc2_upsampled, per (row x psum-tile) frag
                for r in range(7):
                    pr = (7 * sb + r) // 8
                    a = r * 448
                    while a < (r + 1) * 448:
                        t = a // 392
                        b = min((r + 1) * 448, (t + 1) * 392)
                        pc0 = (a - r * 448) // 8
                        w = (b - a) // 8
                        hgv = hgc2[:, pr * 56 + pc0:pr * 56 + pc0 + w, None] \
                            .to_broadcast([128, w, 8])
                        nc.vector.tensor_tensor(
                            h_sb[:, a:b].rearrange("p (c e) -> p c e", e=8),
                            hps[t][:, a - t * 392:b - t * 392]
                            .rearrange("p (c e) -> p c e", e=8),
                            hgv, ALU.add)
                        a = b
                # proj + residual + store; 2 image rows per psum tile.
                # residual split: DVE fused add vs ACT evict + GpSimd add
                for gi, r0 in enumerate(range(0, 7, 2)):
                    nr = min(2, 7 - r0)
                    a = r0 * 448
                    w = nr * 448
                    pp = bpps.tile([128, 2, 512], FP32, tag="pp")
                    for rr in range(nr):
                        nc.tensor.matmul(pp[:, rr, 0:448], W2p[:],
                                         h_sb[:, a + rr * 448:a + rr * 448 + 448],
                                         skip_group_check=True)
                    ot = bout.tile([128, 896], FP32, tag="ot")
                    otv = ot[:, 0:w].rearrange("p (u c) -> p u c", u=nr)
                    xv = XB[:, o + a:o + a + w].rearrange("p (u c) -> p u c",
                                                          u=nr)
                    if gi % 2 == 0:
                        nc.vector.tensor_tensor(otv, pp[:, 0:nr, 0:448], xv,
                                                ALU.add)
                    else:
                        ps = bout.tile([128, 896], FP32, tag="ps")
                        nc.scalar.copy(
                            ps[:, 0:w].rearrange("p (u c) -> p u c", u=nr),
                            pp[:, 0:nr, 0:448])
                        nc.gpsimd.tensor_tensor(ot[:, 0:w], ps[:, 0:w],
                                                XB[:, o + a:o + a + w], ALU.add)
                    nc.sync.dma_start(out=y2[:, o + a:o + a + w], in_=ot[:, 0:w])


def _fold_weights(x, gn_w, gn_b, q_w, q_b, k_w, k_b, v_w, v_b, proj_w):
    """Per-sample folded weight sets for the device program."""
    bf16 = mybir.dt.np(BF16)
    outs = []
    for s in range(B):
        xf = np.ascontiguousarray(x[s]).ravel()
        m = float(xf.mean(dtype=np.float64))
        sq = float(np.dot(xf, xf)) / xf.size
        r = 1.0 / np.sqrt(sq - m * m + EPS)
        scale = (r * gn_w).astype(np.float64)             # [C]
        shift = (gn_b - m * r * gn_w).astype(np.float64)  # [C]

        def fold(w, bias):
            A = (w.astype(np.float64) * scale[None, :])
            c = w.astype(np.float64) @ shift + bias.astype(np.float64)
            return A, c

        Aq, cq = fold(q_w, q_b)
        Ak, ck = fold(k_w, k_b)
        Av, cv = fold(v_w, v_b)

        def bd(a):  # block-diag duplicated transpose [128,128]
            z = np.zeros((128, 128), np.float64)
            z[0:64, 0:64] = a.T
            z[64:128, 64:128] = a.T
            return z

        d = {
            "w2q": (bd(Aq) / 256.0).astype(bf16),
            "w2k": bd(Ak).astype(bf16),
            "w2v": bd(Av).astype(bf16),
            "w2p": bd(proj_w.astype(np.float64)).astype(bf16),
            "cq2": np.tile(cq / 256.0, 2)[:, None].astype(np.float32),
            "ck2": np.tile(ck, 2)[:, None].astype(np.float32),
            "cv2": np.tile(cv, 2)[:, None].astype(np.float32),
            "agq": (Aq.T / (64.0 * 8.0)).astype(bf16),
            "agk": (Ak.T / 64.0).astype(bf16),
            "agv": (Av.T / 64.0).astype(bf16),
            "cgq": (cq[:, None] / 8.0).astype(np.float32),
            "cgk": ck[:, None].astype(np.float32),
        }
        outs.append(d)
    return outs


def _get_runner(reps=1):
    """Compile (once) and return f(concat_inputs: dict) -> y_global [1024, F]."""
    key = ("runner", reps)
    if key in _CACHE:
        return _CACHE[key]

    import jax
    from jax.sharding import Mesh, PartitionSpec
    from jax.experimental.shard_map import shard_map
    from concourse.bass2jax import (_bass_exec_p, install_neuronx_cc_hook,
                                    partition_id_tensor)

    nc = _build_bass(reps)
    install_neuronx_cc_hook()

    partition_name = (nc.partition_id_tensor.name
                      if nc.partition_id_tensor else None)
    in_names, out_names, out_avals = [], [], []
    for alloc in nc.m.functions[0].allocations:
        if not isinstance(alloc, mybir.MemoryLocationSet):
            continue
        name = alloc.memorylocations[0].name
        if alloc.kind == "ExternalInput":
            if name != partition_name:
                in_names.append(name)
        elif alloc.kind == "ExternalOutput":
            out_names.append(name)
            out_avals.append(jax.core.ShapedArray(
                tuple(alloc.tensor_shape), mybir.dt.np(alloc.dtype)))
    n_params = len(in_names)
    all_names = tuple(in_names + out_names +
                      ([partition_name] if partition_name else []))

    def _body(*args):
        operands = list(args)
        if partition_name is not None:
            operands.append(partition_id_tensor())
        return tuple(_bass_exec_p.bind(
            *operands, out_avals=tuple(out_avals), in_names=all_names,
            out_names=tuple(out_names), lowering_input_output_aliases=(),
            sim_require_finite=True, sim_require_nnan=True, nc=nc))

    devices = jax.devices()[:NCORES]
    mesh = Mesh(np.asarray(devices), ("core",))
    nio = n_params + len(out_names)
    sharded = jax.jit(
        shard_map(_body, mesh=mesh, in_specs=(PartitionSpec("core"),) * nio,
                  out_specs=(PartitionSpec("core"),) * len(out_names),
                  check_rep=False),
        keep_unused=True)

    zeros = [jax.device_put(np.zeros((NCORES * a.shape[0],) + a.shape[1:],
                                     a.dtype)) for a in out_avals]

    def run(concat_ins):
        out = sharded(*[concat_ins[n] for n in in_names], *zeros)
        return np.asarray(out[out_names.index("y2")])

    _CACHE[key] = run
    return run


def _marshal(x, folded):
    """Concatenated per-core inputs, in BIR order, as one dict."""
    xg = np.ascontiguousarray(
        x.reshape(B, C, GROUP, 2, 56, SIZE)
        .transpose(0, 2, 3, 1, 4, 5)).reshape(NCORES * 128, F)
    ins = {"x2": xg}
    for name in ("w2q", "w2k", "w2v", "w2p", "cq2", "ck2", "cv2",
                 "agq", "agk", "agv", "cgq", "cgk"):
        ins[name] = np.concatenate(
            [folded[c // GROUP][name] for c in range(NCORES)], axis=0)
    return ins


def _unmarshal(yg):
    return np.ascontiguousarray(
        yg.reshape(B, GROUP, 2, C, 56, SIZE)
        .transpose(0, 3, 1, 2, 4, 5)).reshape(B, C, SIZE, SIZE)


def kernel(x, gn_w, gn_b, q_w, q_b, k_w, k_b, v_w, v_b, proj_w):
    x = np.asarray(x, np.float32)
    args = [np.asarray(a, np.float32) for a in
            (gn_w, gn_b, q_w, q_b, k_w, k_b, v_w, v_b, proj_w)]
    run = _get_runner(1)
    folded = _fold_weights(x, *args)
    yg = run(_marshal(x, folded))
    return _unmarshal(yg)


# revision 26
# speedup vs baseline: 6.0733x; 6.0733x over previous
"""AttnBlock6 Trainium2 kernel (Bass/Tile, 8 NeuronCores).

Math (per sample):
  xn = GroupNorm1(x);  q,k,v = 1x1conv(xn);  wm = softmax(qf^T kf / 256)
  hp = vf @ wm^T   (patch attention over 196-wide chunks, contraction 65536)
  pooled 8x8 -> qg,kg,vg [64,3136]; wg = softmax(qg^T kg / 8); hg = vg @ wg^T
  out = x + proj( 0.75*hp + 0.25*up8(hg) )

Distribution: cores 0-3 -> sample 0, cores 4-7 -> sample 1; each core owns 112
image rows (= 256 patch chunks, 784 pooled positions). Two tiny collectives
per 4-core group: AllReduce of the 196x196 gram matrix, AllGather of pooled x.

Host-side folding: per-sample GroupNorm mean/inv-std are computed on the host
(O(N) numpy) and folded into per-sample q/k/v weight matrices, so the device
never materializes xn. Softmax scales (1/256, 1/8) and the 0.75/0.25 combine
weights are folded into weights/normalizers. v-bias contributes exactly c_v to
the combined h (softmax rows sum to 1), folded in once via the hg term.

Device layout: partition p = 64*half + channel; free = within-half position
(row-major over the core's 56-row half). Block-diagonal duplicated weights
give full K=128 matmul contractions.
"""

import numpy as np

import concourse.bacc as bacc
import concourse.bass as bass
import concourse.tile as tile
from concourse import mybir
from concourse.masks import make_identity

C = 64
SIZE = 448
P2 = 196
TG = 56
POOL = 8
EPS = 1e-5
B = 2

NCORES = 8
GROUP = 4                  # cores per sample
ROWS = SIZE // GROUP       # 112 image rows per core
F = ROWS * SIZE // 2       # 25088 free elems per half
SB = 3136                  # super-block: 7 rows = 16 chunks per half
NSB = F // SB              # 8
PPL = 784                  # pooled positions per core (2 * 7 * 56)
PG = 3136                  # pooled positions per sample (56*56)
KT = 112                   # pooled k-tile (fits rank-contiguous AGed layout)
NKT = PG // KT             # 28

FP32 = mybir.dt.float32
FP32R = mybir.dt.float32r
BF16 = mybir.dt.bfloat16
AF = mybir.ActivationFunctionType
ALU = mybir.AluOpType

_CACHE = {}


def _build_bass(reps=1):
    nc = bacc.Bacc("TRN2", target_bir_lowering=False, debug=False,
                   num_devices=NCORES)

    x2 = nc.dram_tensor("x2", [128, F], BF16, kind="ExternalInput")
    w2q = nc.dram_tensor("w2q", [128, 128], BF16, kind="ExternalInput")
    w2k = nc.dram_tensor("w2k", [128, 128], BF16, kind="ExternalInput")
    w2v = nc.dram_tensor("w2v", [128, 128], BF16, kind="ExternalInput")
    w2p = nc.dram_tensor("w2p", [128, 128], BF16, kind="ExternalInput")
    cq2 = nc.dram_tensor("cq2", [128, 1], FP32, kind="ExternalInput")
    ck2 = nc.dram_tensor("ck2", [128, 1], FP32, kind="ExternalInput")
    cv2 = nc.dram_tensor("cv2", [128, 1], FP32, kind="ExternalInput")
    agq = nc.dram_tensor("agq", [64, 64], BF16, kind="ExternalInput")
    agk = nc.dram_tensor("agk", [64, 64], BF16, kind="ExternalInput")
    agv = nc.dram_tensor("agv", [64, 64], BF16, kind="ExternalInput")
    cgq = nc.dram_tensor("cgq", [64, 1], FP32, kind="ExternalInput")
    cgk = nc.dram_tensor("cgk", [64, 1], FP32, kind="ExternalInput")
    y2 = nc.dram_tensor("y2", [128, F], BF16, kind="ExternalOutput")

    with tile.TileContext(nc) as tc:
        for _ in range(reps):
            _emit(nc, tc, x2, w2q, w2k, w2v, w2p, cq2, ck2, cv2,
                  agq, agk, agv, cgq, cgk, y2)
    nc.finalize()
    return nc


def _ap(t, dims, offset=0):
    """Manual strided view of a DRAM tile: dims = [[step, count], ...]."""
    return bass.AP(tensor=t.tensor, offset=t.offset + offset, ap=list(dims))


def _emit(nc, tc, x2, w2q, w2k, w2v, w2p, cq2, ck2, cv2,
          agq, agk, agv, cgq, cgk, y2):
    import contextlib
    ctx = contextlib.ExitStack()
    with ctx:
        singles = ctx.enter_context(tc.tile_pool(name="singles", bufs=1))
        dram = ctx.enter_context(tc.tile_pool(name="dram", bufs=1, space="DRAM"))

        # ---- load constants ----
        W2q = singles.tile([128, 128], BF16)
        W2k = singles.tile([128, 128], BF16)
        W2v = singles.tile([128, 128], BF16)
        W2p = singles.tile([128, 128], BF16)
        for t, src in ((W2q, w2q), (W2k, w2k), (W2v, w2v), (W2p, w2p)):
            nc.gpsimd.dma_start(out=t[:], in_=src[:, :])
        Cq = singles.tile([128, 1], FP32)
        Ck = singles.tile([128, 1], FP32)
        Cv = singles.tile([128, 1], FP32)
        for t, src in ((Cq, cq2), (Ck, ck2), (Cv, cv2)):
            nc.sync.dma_start(out=t[:], in_=src[:, :])
        Agq = singles.tile([64, 64], BF16)
        Agk = singles.tile([64, 64], BF16)
        Agv = singles.tile([64, 64], BF16)
        for t, src in ((Agq, agq), (Agk, agk), (Agv, agv)):
            nc.sync.dma_start(out=t[:], in_=src[:, :])
        Cgq = singles.tile([64, 1], FP32)
        Cgk = singles.tile([64, 1], FP32)
        for t, src in ((Cgq, cgq), (Cgk, cgk)):
            nc.sync.dma_start(out=t[:], in_=src[:, :])
        ident = singles.tile([98, 98], FP32)
        make_identity(nc, ident)

        # ---- persistent SBUF ----
        XB = singles.tile([128, F], BF16)         # bf16 x, 2-half layout
        poolw = singles.tile([128, 56, 56], FP32)  # w-pooled sums
        pool2 = singles.tile([128, 392], FP32)     # 8x8-pooled sums (2-half)
        pool2b = singles.tile([128, 392], BF16)

        # DRAM scratch / collective buffers
        cc_m_in = dram.tile([P2, P2], BF16)
        cc_m_out = dram.tile([GROUP, P2, P2], BF16)
        cc_p_in = dram.tile([64, PPL], BF16)
        cc_p_out = dram.tile([GROUP * 64, PPL], BF16)
        hg_dram = dram.tile([64, PPL], FP32)
        rec_dram = dram.tile([1, PPL], FP32)

        # =========== PASS A ===========
        mps_a = None
        mps_b = None
        with (
            tc.tile_pool(name="apsum", bufs=3, space="PSUM") as apsum,
            tc.tile_pool(name="mpsum", bufs=1, space="PSUM") as mpsum,
            tc.tile_pool(name="aqk", bufs=4) as aqk,
            tc.tile_pool(name="apool", bufs=2) as apool,
        ):
            mps_a = mpsum.tile([98, P2], FP32, tag="mps_a")
            mps_b = mpsum.tile([98, P2], FP32, tag="mps_b")
            nblk = 0
            for sb in range(NSB):
                o = sb * SB
                xb = XB[:, o:o + SB]
                nc.sync.dma_start(out=xb, in_=x2[:, o:o + SB])
                # pooling (from bf16 x): sb0-5 one DVE strided reduce,
                # sb6-7 gpsimd 3-op tree
                pws = poolw[:, sb * 7:sb * 7 + 7, :]
                if sb < 6:
                    nc.vector.reduce_sum(
                        out=pws,
                        in_=xb.rearrange("p (r c ew) -> p r c ew", r=7, ew=8),
                        axis=mybir.AxisListType.X)
                else:
                    x2v = xb.rearrange("p (q e) -> p q e", e=2)
                    pt = apool.tile([128, 1568], FP32, tag="pt")
                    ptv = pt[:].rearrange("p (q e) -> p q e", e=2)
                    nc.gpsimd.tensor_tensor(pt[:], x2v[:, :, 0], x2v[:, :, 1],
                                            ALU.add)
                    pu = apool.tile([128, 784], FP32, tag="pu")
                    puv = pu[:].rearrange("p (q e) -> p q e", e=2)
                    nc.gpsimd.tensor_tensor(pu[:], ptv[:, :, 0], ptv[:, :, 1],
                                            ALU.add)
                    nc.gpsimd.tensor_tensor(pws, puv[:, :, 0], puv[:, :, 1],
                                            ALU.add)
                # q/k projections + gram accumulation (784-col blocks)
                for j in range(SB // 784):
                    a = o + j * 784
                    qp = apsum.tile([128, 2, 512], FP32, tag="qkp")
                    kp = apsum.tile([128, 2, 512], FP32, tag="qkp")
                    for u in range(2):
                        xr = XB[:, a + u * 392:a + u * 392 + 392]
                        nc.tensor.matmul(qp[:, u, 0:392], W2q[:], xr,
                                         skip_group_check=True)
                        nc.tensor.matmul(kp[:, u, 0:392], W2k[:], xr,
                                         skip_group_check=True)
                    qs = aqk.tile([128, 784], BF16, tag="qs")
                    ks = aqk.tile([128, 784], BF16, tag="ks")
                    qsv = qs[:].rearrange("p (u c) -> p u c", u=2)
                    ksv = ks[:].rearrange("p (u c) -> p u c", u=2)
                    if j % 2 == 0:
                        nc.scalar.activation(qsv, qp[:, :, 0:392], AF.Identity,
                                             bias=Cq[:, 0:1])
                        nc.vector.tensor_scalar(ksv, kp[:, :, 0:392],
                                                Ck[:, 0:1], None, op0=ALU.add)
                    else:
                        nc.vector.tensor_scalar(qsv, qp[:, :, 0:392],
                                                Cq[:, 0:1], None, op0=ALU.add)
                        nc.scalar.activation(ksv, kp[:, :, 0:392], AF.Identity,
                                             bias=Ck[:, 0:1])
                    for cc in range(4):
                        q1 = qs[:, cc * 196:cc * 196 + 196]
                        k1 = ks[:, cc * 196:cc * 196 + 196]
                        first = nblk == 0
                        last = nblk == 127
                        nc.tensor.matmul(mps_a[:], q1[:, 0:98], k1,
                                         start=first, stop=last)
                        nc.tensor.matmul(mps_b[:], q1[:, 98:196], k1,
                                         start=first, stop=last)
                        nblk += 1
            # h-pool on gpsimd (3-op tree) + ship pooled x -> AllGather
            hv = poolw[:].rearrange("p (r e) c -> p r e c", e=2)
            ht = apool.tile([128, 28, 56], FP32, tag="pt")
            nc.gpsimd.tensor_tensor(ht[:], hv[:, :, 0, :], hv[:, :, 1, :],
                                    ALU.add)
            htv = ht[:].rearrange("p (r e) c -> p r e c", e=2)
            hu = apool.tile([128, 14, 56], FP32, tag="pu")
            nc.gpsimd.tensor_tensor(hu[:], htv[:, :, 0, :], htv[:, :, 1, :],
                                    ALU.add)
            huv = hu[:].rearrange("p (r e) c -> p r e c", e=2)
            nc.gpsimd.tensor_tensor(
                pool2[:].rearrange("p (r c) -> p r c", r=7),
                huv[:, :, 0, :], huv[:, :, 1, :], ALU.add)
            nc.gpsimd.tensor_copy(out=pool2b[:], in_=pool2[:])
            nc.gpsimd.dma_start(
                out=_ap(cc_p_in, [[392, 2], [PPL, 64], [1, 392]]),
                in_=pool2b[:])
            nc.gpsimd.collective_compute(
                "AllGather", ALU.bypass,
                replica_groups=[[0, 1, 2, 3], [4, 5, 6, 7]],
                ins=[cc_p_in.opt()], outs=[cc_p_out.opt()])

            # gram eviction -> AllReduce
            msb_a = aqk.tile([98, P2], BF16, tag="msb_a")
            msb_b = aqk.tile([98, P2], BF16, tag="msb_b")
            nc.scalar.copy(msb_a[:], mps_a[:])
            nc.scalar.copy(msb_b[:], mps_b[:])
            nc.gpsimd.dma_start(out=cc_m_in[0:98, :], in_=msb_a[:])
            nc.gpsimd.dma_start(out=cc_m_in[98:196, :], in_=msb_b[:])
            nc.gpsimd.collective_compute(
                "AllGather", ALU.bypass,
                replica_groups=[[0, 1, 2, 3], [4, 5, 6, 7]],
                ins=[cc_m_in.opt()], outs=[cc_m_out.opt()])

        # =========== global pooled attention ===========
        hgc2 = singles.tile([128, 392], FP32)
        with (
            tc.tile_pool(name="gp", bufs=1) as gp,
            tc.tile_pool(name="gwge", bufs=3) as gwge,
            tc.tile_pool(name="gps", bufs=2, space="PSUM") as gps,
            tc.tile_pool(name="ghps", bufs=1, space="PSUM") as ghps,
        ):
            kgsrc = gp.tile([64, PG], BF16, tag="kgsrc")
            nc.sync.dma_start(
                out=kgsrc[:].rearrange("c (r i) -> c r i", r=GROUP),
                in_=_ap(cc_p_out, [[PPL, 64], [64 * PPL, GROUP], [1, PPL]]))
            # qg for my local positions (pooled x back in [64, 784] layout)
            qsrc = gp.tile([64, PPL], BF16, tag="qsrc")
            nc.sync.dma_start(out=qsrc[:], in_=cc_p_in[:, :])
            qgb = gp.tile([64, PPL], BF16, tag="qgb")
            for h in range(2):
                qp = gps.tile([64, 392], FP32, tag="gsm")
                nc.tensor.matmul(qp[:], Agq[:],
                                 qsrc[:, h * 392:h * 392 + 392])
                nc.scalar.activation(qgb[:, h * 392:h * 392 + 392], qp[:],
                                     AF.Identity, bias=Cgq[:, 0:1])
            # kg for all positions of the sample
            kgb = gp.tile([64, PG], BF16, tag="kgb")
            for j in range(2 * GROUP):
                kp = gps.tile([64, 392], FP32, tag="gsm")
                nc.tensor.matmul(kp[:], Agk[:], kgsrc[:, j * 392:(j + 1) * 392])
                nc.scalar.activation(kgb[:, j * 392:(j + 1) * 392], kp[:],
                                     AF.Identity, bias=Cgk[:, 0:1])
            # vgT tiles [112, 65] (last col = ones for the denominator row)
            vgT = gp.tile([112, NKT, 65], BF16, tag="vgT")
            for kt in range(NKT):
                vp = gps.tile([112, 64], FP32, tag="gsm")
                nc.tensor.matmul(vp[:], kgsrc[:, kt * KT:(kt + 1) * KT], Agv[:])
                nc.scalar.copy(vgT[:, kt, 0:64], vp[:])
                nc.vector.memset(vgT[:, kt, 64:65], 1.0)
            # stream k-tiles: wgT logits -> exp -> accumulate hg + denom
            hg0 = ghps.tile([65, 392], FP32, tag="hg0")
            hg1 = ghps.tile([65, 392], FP32, tag="hg1")
            for kt in range(NKT):
                wgp = gps.tile([112, PPL], FP32, tag="wgp")
                for n in range(2):
                    nc.tensor.matmul(wgp[:, n * 392:n * 392 + 392],
                                     kgb[:, kt * KT:(kt + 1) * KT],
                                     qgb[:, n * 392:n * 392 + 392])
                wge = gwge.tile([112, PPL], BF16, tag="wge")
                nc.scalar.activation(wge[:], wgp[:], AF.Exp)
                for n, hgp in ((0, hg0), (1, hg1)):
                    nc.tensor.matmul(hgp[:], vgT[:, kt, :],
                                     wge[:, n * 392:n * 392 + 392],
                                     start=(kt == 0), stop=(kt == NKT - 1))
            # normalize + rearrange to (half,c) layout via DRAM bounce
            hgA = gp.tile([64, PPL], FP32, tag="hgA")
            rec = gp.tile([1, PPL], FP32, tag="rec")
            for n, hgp in ((0, hg0), (1, hg1)):
                nc.scalar.copy(hgA[:, n * 392:n * 392 + 392], hgp[0:64, :])
                nc.scalar.copy(rec[:, n * 392:n * 392 + 392], hgp[64:65, :])
            nc.vector.reciprocal(out=rec[:], in_=rec[:])
            nc.vector.tensor_scalar_mul(rec[:], rec[:], 0.25)
            nc.sync.dma_start(out=hg_dram[:], in_=hgA[:])
            nc.sync.dma_start(out=rec_dram[:], in_=rec[:])
            hgB = gp.tile([128, 392], FP32, tag="hgB")
            nc.sync.dma_start(out=hgB[:],
                              in_=_ap(hg_dram, [[392, 2], [PPL, 64], [1, 392]]))
            rec2 = gp.tile([128, 392], FP32, tag="rec2")
            nc.sync.dma_start(
                out=rec2[:],
                in_=_ap(rec_dram, [[392, 2], [0, 64], [1, 392]]))
            nc.vector.tensor_tensor(hgc2[:], hgB[:], rec2[:], ALU.mult)
            nc.vector.tensor_scalar(hgc2[:], hgc2[:], Cv[:, 0:1], None,
                                    op0=ALU.add)

        # =========== local softmax of wm; build wmT (bf16, x0.75) ===========
        wmT_a = singles.tile([98, P2], BF16)   # k in [0,98)
        wmT_b = singles.tile([98, P2], BF16)   # k in [98,196)
        with (
            tc.tile_pool(name="wmp", bufs=1) as wmp,
            tc.tile_pool(name="wmps", bufs=2, space="PSUM") as wmps,
        ):
            m0 = wmp.tile([98, GROUP, P2], BF16, tag="m0")
            m1 = wmp.tile([98, GROUP, P2], BF16, tag="m1")
            nc.sync.dma_start(
                out=m0[:], in_=_ap(cc_m_out, [[P2, 98], [P2 * P2, GROUP],
                                              [1, P2]]))
            nc.sync.dma_start(
                out=m1[:], in_=_ap(cc_m_out, [[P2, 98], [P2 * P2, GROUP],
                                              [1, P2]], offset=98 * P2))
            ms0 = wmp.tile([98, P2], FP32, tag="ms0")
            ms1 = wmp.tile([98, P2], FP32, tag="ms1")
            nc.vector.tensor_tensor(ms0[:], m0[:, 0, :], m0[:, 1, :], ALU.add)
            nc.vector.tensor_tensor(ms0[:], ms0[:], m0[:, 2, :], ALU.add)
            nc.vector.tensor_tensor(ms0[:], ms0[:], m0[:, 3, :], ALU.add)
            nc.vector.tensor_tensor(ms1[:], m1[:, 0, :], m1[:, 1, :], ALU.add)
            nc.vector.tensor_tensor(ms1[:], ms1[:], m1[:, 2, :], ALU.add)
            nc.vector.tensor_tensor(ms1[:], ms1[:], m1[:, 3, :], ALU.add)
            e0 = wmp.tile([98, P2], FP32, tag="e0")
            e1 = wmp.tile([98, P2], FP32, tag="e1")
            nc.scalar.activation(e0[:], ms0[:], AF.Exp)
            nc.scalar.activation(e1[:], ms1[:], AF.Exp)
            s0 = wmp.tile([98, 1], FP32, tag="s0")
            s1 = wmp.tile([98, 1], FP32, tag="s1")
            nc.vector.reduce_sum(out=s0[:], in_=e0[:], axis=mybir.AxisListType.X)
            nc.vector.reduce_sum(out=s1[:], in_=e1[:], axis=mybir.AxisListType.X)
            nc.vector.reciprocal(out=s0[:], in_=s0[:])
            nc.vector.reciprocal(out=s1[:], in_=s1[:])
            nc.vector.tensor_scalar_mul(s0[:], s0[:], 0.75)
            nc.vector.tensor_scalar_mul(s1[:], s1[:], 0.75)
            wn0 = wmp.tile([98, P2], FP32, tag="wn0")
            wn1 = wmp.tile([98, P2], FP32, tag="wn1")
            nc.vector.tensor_scalar(wn0[:], e0[:], s0[:, 0:1], None, op0=ALU.mult)
            nc.vector.tensor_scalar(wn1[:], e1[:], s1[:, 0:1], None, op0=ALU.mult)
            for (dst, srcs) in ((wmT_a, (wn0[:, 0:98], wn1[:, 0:98])),
                                (wmT_b, (wn0[:, 98:196], wn1[:, 98:196]))):
                for half, src in enumerate(srcs):
                    tp = wmps.tile([98, 98], FP32, tag="tp")
                    nc.tensor.transpose(tp[:], src, ident[:])
                    nc.scalar.copy(dst[:, half * 98:half * 98 + 98], tp[:])

        # =========== PASS B ===========
        with (
            tc.tile_pool(name="bvt", bufs=4) as bvt,
            tc.tile_pool(name="bh", bufs=2) as bh,
            tc.tile_pool(name="bout", bufs=4) as bout,
            tc.tile_pool(name="bvps", bufs=2, space="PSUM") as bvps,
            tc.tile_pool(name="bhps", bufs=2, space="PSUM") as bhps,
            tc.tile_pool(name="bpps", bufs=2, space="PSUM") as bpps,
        ):
            for sb in range(NSB):
                o = sb * SB
                h_sb = bh.tile([128, SB], BF16, tag="h")
                hps = []
                for t in range(SB // 392):
                    a = t * 392
                    vp = bvps.tile([98, 512], FP32, tag="vp")
                    for u in range(4):
                        nc.tensor.matmul(vp[:, u * 128:u * 128 + 128],
                                         XB[:, o + a + u * 98:o + a + u * 98 + 98],
                                         W2v[:], skip_group_check=True)
                    vt = bvt.tile([98, 512], BF16, tag="vt")
                    nc.scalar.copy(vt[:], vp[:])
                    hp = bhps.tile([128, 392], FP32, tag="hp")
                    for cc in range(2):
                        u = 2 * cc
                        nc.tensor.matmul(hp[:, cc * 196:cc * 196 + 196],
                                         vt[:, u * 128:u * 128 + 128], wmT_a[:],
                                         start=True, stop=False,
                                         skip_group_check=True)
                        nc.tensor.matmul(hp[:, cc * 196:cc * 196 + 196],
                                         vt[:, (u + 1) * 128:(u + 1) * 128 + 128],
                                         wmT_b[:],
                                         start=False, stop=True,
                                         skip_group_check=True)
                    hps.append(hp)
                # combine h = hp + hgc2_upsampled, per (row x psum-tile) frag
                for r in range(7):
                    pr = (7 * sb + r) // 8
                    a = r * 448
                    while a < (r + 1) * 448:
                        t = a // 392
                        b = min((r + 1) * 448, (t + 1) * 392)
                        pc0 = (a - r * 448) // 8
                        w = (b - a) // 8
                        hgv = hgc2[:, pr * 56 + pc0:pr * 56 + pc0 + w, None] \
                            .to_broadcast([128, w, 8])
                        nc.vector.tensor_tensor(
                            h_sb[:, a:b].rearrange("p (c e) -> p c e", e=8),
                            hps[t][:, a - t * 392:b - t * 392]
                            .rearrange("p (c e) -> p c e", e=8),
                            hgv, ALU.add)
                        a = b
                # proj + residual + store; 2 image rows per psum tile.
                # residual split: DVE fused add vs ACT evict + GpSimd add
                for gi, r0 in enumerate(range(0, 7, 2)):
                    nr = min(2, 7 - r0)
                    a = r0 * 448
                    w = nr * 448
                    pp = bpps.tile([128, 2, 512], FP32, tag="pp")
                    for rr in range(nr):
                        nc.tensor.matmul(pp[:, rr, 0:448], W2p[:],
                                         h_sb[:, a + rr * 448:a + rr * 448 + 448],
                                         skip_group_check=True)
                    ot = bout.tile([128, 896], BF16, tag="ot")
                    otv = ot[:, 0:w].rearrange("p (u c) -> p u c", u=nr)
                    xv = XB[:, o + a:o + a + w].rearrange("p (u c) -> p u c",
                                                          u=nr)
                    if gi % 2 == 0:
                        nc.vector.tensor_tensor(otv, pp[:, 0:nr, 0:448], xv,
                                                ALU.add)
                    else:
                        ps = bout.tile([128, 896], BF16, tag="ps")
                        nc.scalar.copy(
                            ps[:, 0:w].rearrange("p (u c) -> p u c", u=nr),
                            pp[:, 0:nr, 0:448])
                        nc.gpsimd.tensor_tensor(ot[:, 0:w], ps[:, 0:w],
                                                XB[:, o + a:o + a + w], ALU.add)
                    nc.sync.dma_start(out=y2[:, o + a:o + a + w], in_=ot[:, 0:w])


def _fold_weights(x, gn_w, gn_b, q_w, q_b, k_w, k_b, v_w, v_b, proj_w):
    """Per-sample folded weight sets for the device program."""
    bf16 = mybir.dt.np(BF16)
    outs = []
    for s in range(B):
        xf = np.ascontiguousarray(x[s]).ravel()
        m = float(xf.mean(dtype=np.float64))
        sq = float(np.dot(xf, xf)) / xf.size
        r = 1.0 / np.sqrt(sq - m * m + EPS)
        scale = (r * gn_w).astype(np.float64)             # [C]
        shift = (gn_b - m * r * gn_w).astype(np.float64)  # [C]

        def fold(w, bias):
            A = (w.astype(np.float64) * scale[None, :])
            c = w.astype(np.float64) @ shift + bias.astype(np.float64)
            return A, c

        Aq, cq = fold(q_w, q_b)
        Ak, ck = fold(k_w, k_b)
        Av, cv = fold(v_w, v_b)

        def bd(a):  # block-diag duplicated transpose [128,128]
            z = np.zeros((128, 128), np.float64)
            z[0:64, 0:64] = a.T
            z[64:128, 64:128] = a.T
            return z

        d = {
            "w2q": (bd(Aq) / 256.0).astype(bf16),
            "w2k": bd(Ak).astype(bf16),
            "w2v": bd(Av).astype(bf16),
            "w2p": bd(proj_w.astype(np.float64)).astype(bf16),
            "cq2": np.tile(cq / 256.0, 2)[:, None].astype(np.float32),
            "ck2": np.tile(ck, 2)[:, None].astype(np.float32),
            "cv2": np.tile(cv, 2)[:, None].astype(np.float32),
            "agq": (Aq.T / (64.0 * 8.0)).astype(bf16),
            "agk": (Ak.T / 64.0).astype(bf16),
            "agv": (Av.T / 64.0).astype(bf16),
            "cgq": (cq[:, None] / 8.0).astype(np.float32),
            "cgk": ck[:, None].astype(np.float32),
        }
        outs.append(d)
    return outs


def _get_runner(reps=1):
    """Compile (once) and return f(concat_inputs: dict) -> y_global [1024, F]."""
    key = ("runner", reps)
    if key in _CACHE:
        return _CACHE[key]

    import jax
    from jax.sharding import Mesh, PartitionSpec
    from jax.experimental.shard_map import shard_map
    from concourse.bass2jax import (_bass_exec_p, install_neuronx_cc_hook,
                                    partition_id_tensor)

    nc = _build_bass(reps)
    install_neuronx_cc_hook()

    partition_name = (nc.partition_id_tensor.name
                      if nc.partition_id_tensor else None)
    in_names, out_names, out_avals = [], [], []
    for alloc in nc.m.functions[0].allocations:
        if not isinstance(alloc, mybir.MemoryLocationSet):
            continue
        name = alloc.memorylocations[0].name
        if alloc.kind == "ExternalInput":
            if name != partition_name:
                in_names.append(name)
        elif alloc.kind == "ExternalOutput":
            out_names.append(name)
            out_avals.append(jax.core.ShapedArray(
                tuple(alloc.tensor_shape), mybir.dt.np(alloc.dtype)))
    n_params = len(in_names)
    all_names = tuple(in_names + out_names +
                      ([partition_name] if partition_name else []))

    def _body(*args):
        operands = list(args)
        if partition_name is not None:
            operands.append(partition_id_tensor())
        return tuple(_bass_exec_p.bind(
            *operands, out_avals=tuple(out_avals), in_names=all_names,
            out_names=tuple(out_names), lowering_input_output_aliases=(),
            sim_require_finite=True, sim_require_nnan=True, nc=nc))

    devices = jax.devices()[:NCORES]
    mesh = Mesh(np.asarray(devices), ("core",))
    nio = n_params + len(out_names)
    sharded = jax.jit(
        shard_map(_body, mesh=mesh, in_specs=(PartitionSpec("core"),) * nio,
                  out_specs=(PartitionSpec("core"),) * len(out_names),
                  check_rep=False),
        keep_unused=True)

    zeros = [jax.device_put(np.zeros((NCORES * a.shape[0],) + a.shape[1:],
                                     a.dtype)) for a in out_avals]

    def run(concat_ins):
        out = sharded(*[concat_ins[n] for n in in_names], *zeros)
        return np.asarray(out[out_names.index("y2")])

    _CACHE[key] = run
    return run


def _marshal(x, folded):
    """Concatenated per-core inputs, in BIR order, as one dict."""
    xg = np.ascontiguousarray(
        x.reshape(B, C, GROUP, 2, 56, SIZE)
        .transpose(0, 2, 3, 1, 4, 5)).reshape(NCORES * 128, F)
    ins = {"x2": xg.astype(mybir.dt.np(BF16))}
    for name in ("w2q", "w2k", "w2v", "w2p", "cq2", "ck2", "cv2",
                 "agq", "agk", "agv", "cgq", "cgk"):
        ins[name] = np.concatenate(
            [folded[c // GROUP][name] for c in range(NCORES)], axis=0)
    return ins


def _unmarshal(yg):
    return np.ascontiguousarray(
        yg.astype(np.float32).reshape(B, GROUP, 2, C, 56, SIZE)
        .transpose(0, 3, 1, 2, 4, 5)).reshape(B, C, SIZE, SIZE)


def kernel(x, gn_w, gn_b, q_w, q_b, k_w, k_b, v_w, v_b, proj_w):
    x = np.asarray(x, np.float32)
    args = [np.asarray(a, np.float32) for a in
            (gn_w, gn_b, q_w, q_b, k_w, k_b, v_w, v_b, proj_w)]
    run = _get_runner(1)
    folded = _fold_weights(x, *args)
    yg = run(_marshal(x, folded))
    return _unmarshal(yg)
